# revision 1
# baseline (speedup 1.0000x reference)
"""MAE attention kernel for Trainium2 (Bass/Tile), data-parallel over 8 NeuronCores.

Problem: B=64, N=197 (14x14+CLS), C=768, H=12 heads, head_dim=64, with a
relative-position-bias table gathered by rp_index and an (all-ones) attn mask.

Strategy per core (8 images each):
  - Host precomputes: x^T per image (concatenated), qkv weight transposes with
    the softmax scale folded into the q rows, the full bias matrix
    rpb_table[rp_index] transposed to [head, k, q] layout, and bf16 casts.
  - Device pipeline (all matmuls bf16 with fp32 PSUM accumulation):
      qk^T  [1536, 197/img] = Wqk @ x^T        (+bias via ones-row matmul)
      v     [197, 768]      = x @ Wv^T          (v bias folded into proj bias)
      s^T   [k, q]          = k_h @ q_h^T        per head, bias added by an
                                                 identity-matmul accumulation
      e^T   = exp(s^T)                           (scalar engine, PSUM source)
      u^T   [64, 197] = v_h^T-stationary AV matmul (unnormalized out^T)
      den   [64, 197] = ones-matrix @ e^T        (denominator broadcast to all
                                                  partitions by the PE itself)
      a^T   = u^T * reciprocal(den)              (DVE, fused with PSUM drain)
      y     [197, 768] = a^T-stationary proj matmul (+proj bias ones-row),
                          DMA'd to DRAM straight from PSUM.
  No transposes, no collectives; outputs gathered on host.
"""

import os
import sys

for _p in ("/opt/trn_rl_repo", "/root/.axon_site/_ro/trn_rl_repo"):
    if os.path.isdir(_p) and _p not in sys.path:
        sys.path.insert(0, _p)

import numpy as np
import ml_dtypes

BF16 = ml_dtypes.bfloat16

B, N, C, H, HD = 64, 197, 768, 12, 64
NCORES = 8
E = B // NCORES          # images per core
SCALE = HD ** -0.5
KT = [128, 69]           # token tiles (197 = 128 + 69)
QT = [128, 69]
WIN = (14, 14)
NUM_REL_DIST = (2 * WIN[0] - 1) * (2 * WIN[1] - 1) + 3

_CACHED = {}


def _split_multi_waits(nc):
    """walrus's per-instruction sync-wait slot is exhausted by Tile's
    kernel-tail drain (it batches every unobserved semaphore onto one CTRL
    instruction). Hoist all but the last wait onto single-wait NoOps."""
    import concourse.mybir as mybir

    n = 0
    for fn in nc.m.functions:
        for bb in fn.blocks:
            out = []
            for inst in bb.instructions:
                si = getattr(inst, "sync_info", None)
                if si is not None and si.on_wait and len(si.on_wait) > 1:
                    for w in si.on_wait[:-1]:
                        nop = mybir.InstNoOp(name=f"I-wsplit-{n}", ins=[], outs=[])
                        n += 1
                        nop.engine = inst.engine
                        nop.sync_info = mybir.SyncInfo(on_wait=[w], on_update=[])
                        out.append(nop)
                    si.on_wait = [si.on_wait[-1]]
                out.append(inst)
            bb.instructions = out
    return n


def _build_program():
    import concourse.bass as bass
    import concourse.mybir as mybir
    from concourse import tile

    f32 = mybir.dt.float32
    bf16 = mybir.dt.bfloat16
    EXP = mybir.ActivationFunctionType.Exp
    CPY = mybir.ActivationFunctionType.Copy
    IDN = mybir.ActivationFunctionType.Identity
    LN = mybir.ActivationFunctionType.Ln

    nc = bass.Bass("TRN2", target_bir_lowering=False, debug=False)

    xT = nc.dram_tensor("xT", [C, E * N], bf16, kind="ExternalInput").ap()
    wqk = nc.dram_tensor("wqk", [C, 2 * C], bf16, kind="ExternalInput").ap()
    bqk = nc.dram_tensor("bqk", [1, 2 * C], bf16, kind="ExternalInput").ap()
    wv = nc.dram_tensor("wv", [C, C], bf16, kind="ExternalInput").ap()
    wp = nc.dram_tensor("wp", [C, C], bf16, kind="ExternalInput").ap()
    pb = nc.dram_tensor("pb", [1, C], bf16, kind="ExternalInput").ap()
    # bias^T packed in head pairs: [6, 2(kt), 128, 394]
    bT = nc.dram_tensor("bT", [H // 2, 2, 128, 2 * N], bf16, kind="ExternalInput").ap()
    ident = nc.dram_tensor("ident", [128, 128], bf16, kind="ExternalInput").ap()
    y = nc.dram_tensor("y", [E, N, C], f32, kind="ExternalOutput").ap()

    with tile.TileContext(nc) as tc:
        import contextlib

        with contextlib.ExitStack() as ctx:
            cpool = ctx.enter_context(tc.tile_pool(name="const", bufs=1))
            ppool = ctx.enter_context(tc.tile_pool(name="ps", bufs=8, space="PSUM"))
            apool = ctx.enter_context(tc.tile_pool(name="act", bufs=1))
            spool = ctx.enter_context(tc.tile_pool(name="scratch", bufs=8))
            xpool_cm = tc.tile_pool(name="xin", bufs=1)
            xpool = xpool_cm.__enter__()

            def load(name, shape, src, dt=bf16, pool=None):
                t = (pool or cpool).tile(shape, dt, tag=name, name=name)
                nc.sync.dma_start(t[:], src)
                return t

            xT_sb = [load(f"xT{k}", [128, E * N], xT[k * 128:(k + 1) * 128, :], pool=xpool) for k in range(6)]
            wqk_sb = [load(f"wqk{k}", [128, 2 * C], wqk[k * 128:(k + 1) * 128, :]) for k in range(6)]
            wv_sb = [load(f"wv{k}", [128, C], wv[k * 128:(k + 1) * 128, :]) for k in range(6)]
            wp_sb = [load(f"wp{k}", [128, C], wp[k * 128:(k + 1) * 128, :]) for k in range(6)]
            bqk_sb = load("bqk", [1, 2 * C], bqk[:, :])
            pb_sb = load("pb", [1, C], pb[:, :])
            id_sb = load("ident", [128, 128], ident[:, :])
            bT_sb = [[load(f"bT{hp}_{kt}", [128, 2 * N], bT[hp, kt, :, :]) for kt in range(2)]
                     for hp in range(H // 2)]
            ones_sb = cpool.tile([128, 128], bf16, tag="ones")
            nc.vector.memset(ones_sb[:], 1.0)
            onesr_sb = cpool.tile([1, 2 * N], bf16, tag="ones_row")
            nc.vector.memset(onesr_sb[:], 1.0)

            # ---- stage 1: qk^T = Wqk @ x^T (+bias), all images, bf16 out ----
            qkT_sb = [cpool.tile([128, E * N], bf16, tag=f"qkT{mt}", name=f"qkT{mt}") for mt in range(12)]
            for mt in range(12):
                pss = [ppool.tile([128, 512], f32, tag="bank", name="ps_qk")
                       for _ in range(E // 2)]
                for pair in range(E // 2):
                    nc.tensor.matmul(pss[pair][:, 0:2 * N],
                                     bqk_sb[0:1, mt * 128:(mt + 1) * 128],
                                     onesr_sb[0:1, :], start=True, stop=False)
                for k in range(6):
                    for pair in range(E // 2):
                        nc.tensor.matmul(
                            pss[pair][:, 0:2 * N],
                            wqk_sb[k][:, mt * 128:(mt + 1) * 128],
                            xT_sb[k][:, pair * 2 * N:(pair + 1) * 2 * N],
                            start=False, stop=(k == 5))
                for pair in range(E // 2):
                    nc.scalar.activation(qkT_sb[mt][:, pair * 2 * N:(pair + 1) * 2 * N],
                                         pss[pair][:, 0:2 * N], CPY)

            # ---- stage 2: v = x @ Wv^T, natural layout, all images ----
            v_sb = [[cpool.tile([128, C], bf16, tag=f"v{e}_{kt}", name=f"v{e}_{kt}") for kt in range(2)]
                    for e in range(E)]
            for e in range(E):
                for kt in range(2):
                    kts = KT[kt]
                    psv = [ppool.tile([128, 512], f32, tag="bank", name="ps_v")
                           for _ in range(2)]
                    for k in range(6):
                        for ch in range(2):
                            nc.tensor.matmul(
                                psv[ch][0:kts, 0:384],
                                xT_sb[k][:, e * N + kt * 128: e * N + kt * 128 + kts],
                                wv_sb[k][:, ch * 384:(ch + 1) * 384],
                                start=(k == 0), stop=(k == 5))
                    for ch in range(2):
                        nc.vector.tensor_copy(
                            v_sb[e][kt][0:kts, ch * 384:(ch + 1) * 384],
                            psv[ch][0:kts, 0:384])
            xpool_cm.__exit__(None, None, None)

            # ---- stage 3: attention per image / head-pair ----
            aT_sb = [[cpool.tile([128, N], bf16, tag=f"aT{e}_{hp}", name=f"aT{e}_{hp}") for hp in range(6)]
                     for e in range(E)]
            def proj_image(e):
                for qt in range(2):
                    qts = QT[qt]
                    psp = [ppool.tile([128, 512], f32, tag="bank", name="ps_p")
                           for _ in range(2)]
                    for ch in range(2):
                        nc.tensor.matmul(psp[ch][0:qts, 0:384], ones_sb[0:1, 0:qts],
                                         pb_sb[0:1, ch * 384:(ch + 1) * 384],
                                         start=True, stop=False)
                    for k in range(6):
                        for ch in range(2):
                            nc.tensor.matmul(
                                psp[ch][0:qts, 0:384],
                                aT_sb[e][k][:, qt * 128: qt * 128 + qts],
                                wp_sb[k][:, ch * 384:(ch + 1) * 384],
                                start=False, stop=(k == 5))
                    for ch in range(2):
                        ysb = spool.tile([128, 384], f32, tag="ysb", name="ysb", bufs=4)
                        if ch == 0:
                            nc.scalar.activation(ysb[0:qts, :], psp[ch][0:qts, 0:384], CPY)
                        else:
                            nc.vector.tensor_copy(ysb[0:qts, :], psp[ch][0:qts, 0:384])
                        nc.sync.dma_start(
                            y[e, qt * 128: qt * 128 + qts, ch * 384:(ch + 1) * 384],
                            ysb[0:qts, :])

            for e in range(E):
                for hp in range(6):
                    expT = []
                    for kt in range(2):
                        kts = KT[kt]
                        ps = ppool.tile([128, 512], f32, tag="bank")
                        for hh in range(2):
                            h = 2 * hp + hh
                            bp = (h % 2) * 64
                            nc.tensor.matmul(
                                ps[0:kts, hh * N:(hh + 1) * N],
                                id_sb[0:kts, 0:kts],
                                bT_sb[hp][kt][0:kts, hh * N:(hh + 1) * N],
                                start=True, stop=False)
                            nc.tensor.matmul(
                                ps[0:kts, hh * N:(hh + 1) * N],
                                qkT_sb[6 + h // 2][bp:bp + 64, e * N + kt * 128: e * N + kt * 128 + kts],
                                qkT_sb[h // 2][bp:bp + 64, e * N:(e + 1) * N],
                                start=False, stop=True)
                        et = spool.tile([128, 2 * N], bf16, tag="expT", bufs=6)
                        nc.scalar.activation(et[0:kts, :], ps[0:kts, 0:2 * N], EXP)
                        expT.append(et)
                    ps_av = ppool.tile([128, 512], f32, tag="bank", name="ps_av")
                    ps_dn = ppool.tile([128, 512], f32, tag="bank", name="ps_dn")
                    for hh in range(2):
                        h = 2 * hp + hh
                        for kt in range(2):
                            kts = KT[kt]
                            nc.tensor.matmul(
                                ps_av[hh * 64:hh * 64 + 64, 0:N],
                                v_sb[e][kt][0:kts, h * 64:(h + 1) * 64],
                                expT[kt][0:kts, hh * N:(hh + 1) * N],
                                start=(kt == 0), stop=(kt == 1))
                            nc.tensor.matmul(
                                ps_dn[hh * 64:hh * 64 + 64, 0:N],
                                ones_sb[0:kts, 0:64],
                                expT[kt][0:kts, hh * N:(hh + 1) * N],
                                start=(kt == 0), stop=(kt == 1))
                    rec = spool.tile([128, N], f32, tag="rec", name="rec", bufs=3)
                    nc.vector.reciprocal(rec[:, :], ps_dn[:, 0:N])
                    nc.vector.tensor_mul(aT_sb[e][hp][:, :], ps_av[:, 0:N], rec[:, :])
                # ---- stage 4 (interleaved): proj + bias for this image ----
                proj_image(e)
    return nc


def _build_rel_pos_index():
    coords = np.stack(np.meshgrid(np.arange(WIN[0]), np.arange(WIN[1]), indexing="ij"))
    cf = coords.reshape(2, -1)
    rel = (cf[:, :, None] - cf[:, None, :]).transpose(1, 2, 0).copy()
    rel[:, :, 0] += WIN[0] - 1
    rel[:, :, 1] += WIN[1] - 1
    rel[:, :, 0] *= 2 * WIN[1] - 1
    idx = np.zeros((N, N), dtype=np.int32)
    idx[1:, 1:] = rel.sum(-1)
    idx[0, 0:] = NUM_REL_DIST - 3
    idx[0:, 0] = NUM_REL_DIST - 2
    idx[0, 0] = NUM_REL_DIST - 1
    return idx


def _host_prep(x, qkv_w, qkv_b, proj_w, proj_b, rpb_table, rp_index):
    """Build the per-core input maps (host-side shards + transposes + casts)."""
    x = np.asarray(x, np.float32)
    qkv_w = np.asarray(qkv_w, np.float32)
    qkv_b = np.asarray(qkv_b, np.float32)
    proj_w = np.asarray(proj_w, np.float32)
    proj_b = np.asarray(proj_b, np.float32)
    rpb_table = np.asarray(rpb_table, np.float32)
    rp_index = np.asarray(rp_index, np.int32)

    wqk_f = qkv_w[0:2 * C, :].copy()
    bqk_f = qkv_b[0:2 * C].copy()
    wqk_f[0:C] *= SCALE
    bqk_f[0:C] *= SCALE
    wqk_h = np.ascontiguousarray(wqk_f.T).astype(BF16)          # [C, 2C]
    bqk_h = bqk_f.reshape(1, 2 * C).astype(BF16)
    wv_h = np.ascontiguousarray(qkv_w[2 * C:3 * C, :].T).astype(BF16)   # [C, C]
    wp_h = np.ascontiguousarray(proj_w.T).astype(BF16)          # [C, C]
    pb_eff = proj_b + proj_w @ qkv_b[2 * C:3 * C]
    pb_h = pb_eff.reshape(1, C).astype(BF16)

    bias = rpb_table[rp_index]                  # [N, N, H] (q, k, h)
    biasT = np.transpose(bias, (2, 1, 0))       # [H, k, q]
    bT_h = np.zeros((H // 2, 2, 128, 2 * N), np.float32)
    for hp in range(H // 2):
        for kt in range(2):
            kts = KT[kt]
            sl = biasT[:, kt * 128: kt * 128 + kts, :]
            bT_h[hp, kt, 0:kts, 0:N] = sl[2 * hp]
            bT_h[hp, kt, 0:kts, N:2 * N] = sl[2 * hp + 1]
    bT_h = bT_h.astype(BF16)
    ident_h = np.eye(128, dtype=np.float32).astype(BF16)

    in_maps = []
    for c in range(NCORES):
        xs = x[c * E:(c + 1) * E]                               # [E, N, C]
        xT_h = np.ascontiguousarray(np.transpose(xs, (2, 0, 1)).reshape(C, E * N)).astype(BF16)
        in_maps.append({
            "xT": xT_h, "wqk": wqk_h, "bqk": bqk_h, "wv": wv_h,
            "wp": wp_h, "pb": pb_h, "bT": bT_h, "ident": ident_h,
        })
    return in_maps


def _get_program():
    if "nc" not in _CACHED:
        _CACHED["nc"] = _build_program()
    return _CACHED["nc"]


def run_on_hw(in_maps, trace=False):
    from concourse.bass_utils import run_bass_kernel_spmd
    nc = _get_program()
    if not _CACHED.get("waits_split"):
        _split_multi_waits(nc)
        _CACHED["waits_split"] = True
    return run_bass_kernel_spmd(nc, in_maps, list(range(NCORES)), trace=trace)


def _reference_numpy(x, qkv_w, qkv_b, proj_w, proj_b, rpb_table, rp_index, attn_mask):
    """Fallback path (exact, host) — only used if attn_mask isn't all ones."""
    x = np.asarray(x, np.float64)
    qkv = x @ np.asarray(qkv_w, np.float64).T + np.asarray(qkv_b, np.float64)
    qkv = qkv.reshape(B, N, 3, H, HD).transpose(2, 0, 3, 1, 4)
    q, k, v = qkv[0], qkv[1], qkv[2]
    attn = np.einsum("bhqd,bhkd->bhqk", q * SCALE, k)
    bias = np.asarray(rpb_table, np.float64)[np.asarray(rp_index)].transpose(2, 0, 1)
    attn = attn + bias[None]
    mask = np.asarray(attn_mask, bool)[:, None, None, :]
    attn = np.where(mask, attn, -np.inf)
    attn = attn - attn.max(-1, keepdims=True)
    ex = np.exp(attn)
    attn = ex / ex.sum(-1, keepdims=True)
    out = np.einsum("bhqk,bhkd->bhqd", attn, v)
    out = out.transpose(0, 2, 1, 3).reshape(B, N, C)
    return (out @ np.asarray(proj_w, np.float64).T + np.asarray(proj_b, np.float64)).astype(np.float32)


def kernel(x, qkv_w, qkv_b, proj_w, proj_b, rpb_table, rp_index, attn_mask):
    attn_mask = np.asarray(attn_mask)
    if not attn_mask.all():
        return _reference_numpy(x, qkv_w, qkv_b, proj_w, proj_b,
                                rpb_table, rp_index, attn_mask)
    in_maps = _host_prep(x, qkv_w, qkv_b, proj_w, proj_b, rpb_table, rp_index)
    res = run_on_hw(in_maps)
    out = np.concatenate([res.results[c]["y"] for c in range(NCORES)], axis=0)
    return out.astype(np.float32)

